# revision 17
# baseline (speedup 1.0000x reference)
"""EnhancedDynamicChannelAttention Trainium2 kernel (bf16 pipeline).

Reference computation (B=16, S=2048, C=1024, H=8, HD=128):
    q[b,h,:]   = pref[b,h]*Wq[:,0] + bq
    k          = f @ Wk.T + bk ;  v = f @ Wv.T + bv       (per head slice)
    scores     = softmax_s(q . k)                          [B,H,S]
    ctx[b,h,:] = sum_s scores * v[b,s,h,:]                 [B,H,HD]
    out        = f + broadcast_s(ctx)

Algebraic folding (exact up to fp reassociation):
  - softmax shift invariance  -> the q.bk term drops entirely.
  - scores[b,h,s] = f[b,s,h,:] . qk[b,h,:]  with  qk = (pref*Wq+bq) @ Wk
  - sum_s attn = 1  ->  ctx = Wv @ (sum_s attn*f[b,s,h,:]) + bv
  So k/v are never materialized.

Precision: the data path runs in bf16 (f storage, score mul, E, PE
matmul inputs, residual add, output store); scores are fp16 (2-byte
for DVE fast modes; fp16 mantissa is plenty for |scores|<~30); PSUM
accumulation and the tail stay fp32.  End-to-end rel err vs the fp64
reference ~4e-3 against the 2e-2 gate.

Distribution: pure data parallel over batch, 2 batches per core.

Engine schedule (DVE is critical -- its program order is kept to
exactly: b0 scores, recip0, b1 scores, recip1, b0 adds, b1 adds):
  A(b): DMA f super tiles (SP ring; qk broadcasts lead the ACT ring) ;
        DVE tmp=f*qk + segmented reduce -> fp16 scores ; ACT exp ->
        bf16 E ; PE uwf/sumE PSUM accumulation.
  T(b): ACT copies uwf from PSUM with 1/sumE scale fused ; PE per-head
        transpose ; ACT gathers the 8 diagonal columns ; ONE PE matmul
        -> ctx8 [8,128] ; Pool +bv -> bf16 ; PE ones-outer-product
        broadcast per head -> PSUM ; ACT -> bf16 SBUF ctx_bc.
        (No DVE ops except the tiny reciprocal.)
  R(b): residual adds split Pool (in-place, early window) / DVE
        (out-of-place staging) ; stores ride the SP ring.
T0 is emitted inside A1 (after its first super tile) so T0's PSUM
copy frees the uwf banks before PE reaches b1's accumulation, and
T1's chain hides under DVE's R0 adds.
"""

import numpy as np

B, S, C = 16, 2048, 1024
H, HD = 8, 128
N_CORES = 8
BPC = B // N_CORES          # batches per core
ST = 2                      # s-rows per partition in a super tile
P = 128
SUP = S // (P * ST)         # super tiles per batch (4)
NT = S // P                 # sub tiles per batch (16)

# residual-add tiles handed to the Pool engine (rest go to DVE)
POOL_HALVES = {0: 3, 1: 1}
# score-mul tiles computed on the Pool engine in its early idle window
POOL_MULS = {0: (6, 7), 1: (6, 7)}

_CACHE = {}


def _build_program():
    import concourse.bass as bass
    import concourse.bacc as bacc
    import concourse.tile as tile
    from concourse import mybir

    f32 = mybir.dt.float32
    f16 = mybir.dt.float16
    bf16 = mybir.dt.bfloat16

    nc = bacc.Bacc("TRN2", debug=False, num_devices=N_CORES)
    f_in = nc.dram_tensor("features", [BPC, S, C], bf16, kind="ExternalInput")
    qk_in = nc.dram_tensor("qkflat", [BPC, C], bf16, kind="ExternalInput")
    wvt_in = nc.dram_tensor("wvt", [HD, HD], bf16, kind="ExternalInput")
    bvf_in = nc.dram_tensor("bvflat", [1, C], bf16, kind="ExternalInput")
    id8_in = nc.dram_tensor("ident8", [8, 8], f32, kind="ExternalInput")
    ones_in = nc.dram_tensor("ones128", [P, 1], bf16, kind="ExternalInput")
    onesrow_in = nc.dram_tensor("onesrow", [1, P], bf16, kind="ExternalInput")
    out_t = nc.dram_tensor("out", [BPC, S, C], bf16, kind="ExternalOutput")

    with tile.TileContext(nc) as tc:
        with (
            tc.tile_pool(name="fpool", bufs=BPC) as fpool,
            tc.tile_pool(name="tmppool", bufs=3) as tmppool,
            tc.tile_pool(name="spool", bufs=2 * SUP) as spool,
            tc.tile_pool(name="small", bufs=2) as small,
            tc.tile_pool(name="ostage", bufs=4) as ostage,
            tc.tile_pool(name="singles", bufs=1) as singles,
            tc.tile_pool(name="ps_uwf", bufs=1, space="PSUM") as ps_uwf,
            tc.tile_pool(name="ps_tail", bufs=1, space="PSUM") as ps_tail,
        ):
            # qk rows first on the SP ring -- they gate the first DVE mul
            qk_bcs = []
            for b in range(BPC):
                qk_bc = small.tile([P, C], bf16, tag="qkbc")
                nc.sync.dma_start(
                    out=qk_bc, in_=qk_in[b : b + 1, :].to_broadcast([P, C])
                )
                qk_bcs.append(qk_bc)

            wvt_sb = singles.tile([HD, HD], bf16)
            nc.scalar.dma_start(out=wvt_sb, in_=wvt_in[:, :])
            bvf_sb = singles.tile([1, C], bf16)
            nc.scalar.dma_start(out=bvf_sb, in_=bvf_in[:, :])
            one1_sb = singles.tile([1, 1], bf16)
            nc.vector.memset(one1_sb, 1.0)
            id8_sb = singles.tile([8, 8], f32)
            nc.scalar.dma_start(out=id8_sb, in_=id8_in[:, :])
            ones_sb = singles.tile([P, 1], bf16)
            nc.scalar.dma_start(out=ones_sb, in_=ones_in[:, :])
            onesrow_sb = singles.tile([1, P], bf16)
            nc.scalar.dma_start(out=onesrow_sb, in_=onesrow_in[:, :])

            fbs = [None] * BPC
            uwfs = [None] * BPC
            sums = [None] * BPC
            recips = [None] * BPC
            ctxs = [None] * BPC

            def scores_super_tile(b, st, fb, qk_bc3, uwfA, uwfB, sumE):
                lo = st * ST
                fview = f_in[b].rearrange("(st p t) c -> st p t c", p=P, t=ST)
                nc.sync.dma_start(out=fb[:, lo : lo + ST, :], in_=fview[st])
                tmp = tmppool.tile([P, ST, C], bf16, tag="tmp")
                if st in POOL_MULS[b]:
                    nc.gpsimd.tensor_mul(tmp, fb[:, lo : lo + ST, :], qk_bc3)
                else:
                    nc.vector.tensor_mul(tmp, fb[:, lo : lo + ST, :], qk_bc3)
                # segmented reduce over d=128 done as two bf16 tree-fold
                # adds (DVE 2x mode) + a short X reduce over 32 -- much
                # faster than one reduce over 128 (which runs at ~1x)
                tmpv = tmp.rearrange("p t (h d) -> p t h d", h=H)
                f64t = spool.tile([P, ST, H, 64], bf16, tag="fold64")
                nc.vector.tensor_add(f64t, tmpv[:, :, :, 0:64], tmpv[:, :, :, 64:128])
                f32t = spool.tile([P, ST, H, 32], bf16, tag="fold32")
                nc.vector.tensor_add(f32t, f64t[:, :, :, 0:32], f64t[:, :, :, 32:64])
                scores = spool.tile([P, ST, H], f16, tag="scores")
                with nc.allow_low_precision(
                    reason="fp16 scores: |s|<30; bf16 folds avg out"
                ):
                    nc.vector.reduce_sum(
                        scores, f32t, axis=mybir.AxisListType.X,
                    )
                E_sup = spool.tile([P, ST, H], bf16, tag="esup")
                nc.scalar.activation(
                    out=E_sup.rearrange("p t h -> p (t h)"),
                    in_=scores.rearrange("p t h -> p (t h)"),
                    func=mybir.ActivationFunctionType.Exp,
                )
                for t in range(ST):
                    first = st == 0 and t == 0
                    last = st == SUP - 1 and t == ST - 1
                    e_sl = E_sup[:, t, :]
                    f_sl = fb[:, lo + t, :]
                    nc.tensor.matmul(sumE, e_sl, ones_sb, start=first, stop=last)
                    nc.tensor.matmul(
                        uwfA[0:8, :], e_sl, f_sl[:, 0:512],
                        start=first, stop=last,
                    )
                    nc.tensor.matmul(
                        uwfB[0:8, :], e_sl, f_sl[:, 512:1024],
                        start=first, stop=last,
                    )

            def phase_scores(b, mid_cb=None):
                """Loads + scores + uwf/sumE accumulation for batch b.
                mid_cb (if given) is emitted after the first super tile so
                its ACT/PE ops land early in those engines' queues."""
                qk_bc3 = qk_bcs[b].rearrange(
                    "p (o c) -> p o c", o=1
                ).broadcast_to([P, ST, C])
                uwfA = ps_uwf.tile([P, 512], f32, tag="uwfA")
                uwfB = ps_uwf.tile([P, 512], f32, tag="uwfB")
                sumE = ps_uwf.tile([8, 1], f32, tag="sumE")
                uwfs[b] = (uwfA, uwfB)
                sums[b] = sumE
                fb = fpool.tile([P, NT, C], bf16, tag="fb")
                fbs[b] = fb
                for st in range(SUP):
                    scores_super_tile(b, st, fb, qk_bc3, uwfA, uwfB, sumE)
                    if st == 0 and mid_cb is not None:
                        mid_cb()

            def tail_recip(b):
                recip = small.tile([8, 1], f32, tag="recip")
                nc.vector.reciprocal(recip, sums[b])
                recips[b] = recip

            def tail_ctx(b):
                """ctx8 (+bv on Pool) -> broadcast bf16 SBUF tile.  No DVE."""
                uwfA, uwfB = uwfs[b]
                recip = recips[b]
                # PSUM -> SBUF with the 1/sumE row scale fused into the copy
                uwf_sb = small.tile([8, C], f32, tag="uwfsb", bufs=1)
                nc.scalar.activation(
                    out=uwf_sb[:, 0:512], in_=uwfA[0:8, :],
                    func=mybir.ActivationFunctionType.Copy, scale=recip,
                )
                nc.scalar.activation(
                    out=uwf_sb[:, 512:1024], in_=uwfB[0:8, :],
                    func=mybir.ActivationFunctionType.Copy, scale=recip,
                )
                # per-head PE transpose into [128, 8*8]; diagonal columns
                # (stride 9) hold wfT[d, h] = uwf[h, h*128+d] / sumE[h]
                # group h occupies cols h*9 .. h*9+7, so its diagonal
                # column (row h) sits at col 10*h -- constant stride 10
                wfT8_ps = ps_tail.tile([P, H * 10], f32, tag="wft8")
                for h in range(H):
                    nc.tensor.transpose(
                        wfT8_ps[:, h * 9 : h * 9 + H],
                        uwf_sb[:, h * HD : (h + 1) * HD],
                        id8_sb,
                    )
                wfd_sb = small.tile([P, H], bf16, tag="wfd", bufs=1)
                nc.scalar.copy(
                    out=wfd_sb,
                    in_=wfT8_ps.rearrange("p (h n) -> p h n", n=10)[:, :, 0],
                )
                # ctx row [1, C]: per head ctx[0, h*HD:] = wfd[:, h] . WvT,
                # then bv folded in via two K=1 ones ⊗ bv accumulate matmuls
                ctx_ps = ps_tail.tile([1, C], f32, tag="ctxrow")
                for half in range(2):
                    cs = slice(half * 512, (half + 1) * 512)
                    nc.tensor.matmul(
                        ctx_ps[0:1, cs], one1_sb, bvf_sb[:, cs],
                        start=True, stop=False, skip_group_check=True,
                    )
                for h in range(H):
                    nc.tensor.matmul(
                        ctx_ps[0:1, h * HD : (h + 1) * HD],
                        wfd_sb[:, h : h + 1],
                        wvt_sb,
                        start=False,
                        stop=True,
                        skip_group_check=True,
                    )
                ctx_row = small.tile([1, C], bf16, tag="ctxrowsb", bufs=1)
                nc.scalar.copy(out=ctx_row, in_=ctx_ps)
                # broadcast down partitions on the PE: ones ⊗ ctx_row
                ctx_bc_ps = ps_tail.tile([P, C], f32, tag="ctxbcps")
                for half in range(2):
                    cs = slice(half * 512, (half + 1) * 512)
                    nc.tensor.matmul(
                        ctx_bc_ps[:, cs], onesrow_sb, ctx_row[:, cs],
                        start=True, stop=True,
                    )
                ctx_bc = small.tile([P, C], bf16, tag="ctxbc")
                nc.scalar.copy(out=ctx_bc, in_=ctx_bc_ps)
                ctxs[b] = ctx_bc

            def resid_halves(b):
                fb = fbs[b]
                oview = out_t[b].rearrange("(st p t) c -> st p t c", p=P, t=ST)
                for st in range(SUP):
                    lo = st * ST
                    yield st, fb[:, lo : lo + ST, :], oview[st]

            def resid_pool(b):
                """Pool-engine in-place adds + stores for its share."""
                ctx_bc2 = ctxs[b].rearrange("p (o c) -> p o c", o=1).broadcast_to(
                    [P, ST, C]
                )
                for idx, fsl, osl in resid_halves(b):
                    if idx >= POOL_HALVES[b]:
                        break
                    ost = ostage.tile([P, ST, C], bf16, tag="oslp", bufs=2)
                    nc.gpsimd.tensor_add(ost, fsl, ctx_bc2)
                    nc.scalar.dma_start(out=osl, in_=ost)

            def resid_dve(b):
                """DVE out-of-place adds + stores for the non-Pool share."""
                ctx_bc2 = ctxs[b].rearrange("p (o c) -> p o c", o=1).broadcast_to(
                    [P, ST, C]
                )
                for idx, fsl, osl in resid_halves(b):
                    if idx < POOL_HALVES[b]:
                        continue
                    ost = ostage.tile([P, ST, C], bf16, tag="osl")
                    nc.vector.tensor_add(ost, fsl, ctx_bc2)
                    nc.sync.dma_start(out=osl, in_=ost)

            phase_scores(0)
            tail_recip(0)
            # T0 is emitted just after b1's first super tile: its ACT/PE ops
            # land before b1's later exps/matmuls in those queues, freeing
            # the uwf PSUM banks early; DVE itself stays on b1's scores.
            phase_scores(1, mid_cb=lambda: tail_ctx(0))
            tail_recip(1)
            resid_pool(0)
            resid_dve(0)
            tail_ctx(1)
            resid_pool(1)
            resid_dve(1)

    nc.finalize()
    return nc


def _get_program():
    if "nc" not in _CACHE:
        _CACHE["nc"] = _build_program()
    return _CACHE["nc"]


def _prep_in_maps(features, preference, Wq, bq, Wk, Wv, bv):
    import ml_dtypes

    f32 = np.float32
    bf16 = ml_dtypes.bfloat16
    # qk[b,h,:] = (pref[b,h]*Wq[:,0] + bq) @ Wk   -> flat [B, C]
    q = preference[:, :, None] * Wq[:, 0][None, None, :] + bq  # [B,H,HD]
    qk = np.einsum("bhe,ed->bhd", q, Wk)  # [B,H,HD]
    qkflat = np.ascontiguousarray(qk.reshape(B, C)).astype(bf16)
    wvt = np.ascontiguousarray(Wv.T).astype(bf16)
    bvflat = np.ascontiguousarray(np.tile(bv, H)[None, :]).astype(bf16)
    id8 = np.eye(8, dtype=f32)
    ones128 = np.ones([P, 1], dtype=bf16)
    onesrow = np.ones([1, P], dtype=bf16)
    fbf = np.ascontiguousarray(features).astype(bf16)

    in_maps = []
    for i in range(N_CORES):
        sl = slice(i * BPC, (i + 1) * BPC)
        in_maps.append(
            {
                "features": fbf[sl],
                "qkflat": qkflat[sl],
                "wvt": wvt,
                "bvflat": bvflat,
                "ident8": id8,
                "ones128": ones128,
                "onesrow": onesrow,
            }
        )
    return in_maps


def kernel(features, preference, Wq, bq, Wk, bk, Wv, bv, **_ignored):
    features = np.asarray(features, dtype=np.float32)
    preference = np.asarray(preference, dtype=np.float32)
    Wq = np.asarray(Wq, dtype=np.float32)
    bq = np.asarray(bq, dtype=np.float32)
    Wk = np.asarray(Wk, dtype=np.float32)
    Wv = np.asarray(Wv, dtype=np.float32)
    bv = np.asarray(bv, dtype=np.float32)

    from concourse.bass_utils import run_bass_kernel_spmd

    nc = _get_program()
    in_maps = _prep_in_maps(features, preference, Wq, bq, Wk, Wv, bv)
    res = run_bass_kernel_spmd(nc, in_maps, core_ids=list(range(N_CORES)))
    out = np.concatenate([r["out"] for r in res.results], axis=0)
    return out.astype(np.float32)
